# revision 1
# baseline (speedup 1.0000x reference)
"""BasePNARetriever Trainium2 kernel (8 NeuronCores, SPMD).

Strategy (v2):
  - Vocab-shard the big embedding @ W_down.T matmul: each core streams a
    [4096, 4096] (host-transposed, padded) slice of text_embeddings and
    computes RtabT[64, 4096] on PE, accumulating in PSUM over 32 K-chunks.
  - PE-transpose RtabT back to row-major and emit a bf16 table with each
    row packed as [val(64) | val^2(64)] (256B rows): the squares ride along
    for free in the gather, eliminating the per-chunk ACT square pass.
    AllGather the 1MB local slice into the full rtab2[32768, 128] bf16.
  - Each core dma_gathers its (padded) rows x 16 tokens = 100352 vectors of
    256B from rtab2, round-robining the 4 SWDGE queues so descriptor
    generation runs on all four Q7 core-pairs concurrently (the Q7 desc-gen
    at ~8ns/desc is otherwise the phase C wall).  Pooling runs as chunk-wide
    bf16 binary trees on DVE (sum+sumsq fused over the packed 128-elem
    rows), per-row scalar finals split between ACT (scale-activations) and
    DVE, small FC on PE via PE-transposed bf16 features, L2-normalize.
  - Host precomputes lengths/log-scales (includes a global mean over all
    50000 rows) and patches the rare rows containing id==0 tokens (the
    device path ignores the valid-token mask; ~25 rows in 50000).

Sync-architecture notes (walrus limits): a Matmult may carry at most ONE
sync wait; HWDGE (nc.sync) DMAs are also tightly limited; DVE/ACT/Pool
queue instructions tolerate several.  Hence: emb chunk loads go through
SWDGE (nc.gpsimd), every PE dummy/absorber reads only wdt_sb (whose DMA
lane PE observes on its first matmul), an extra junk matmul into the psA
pad columns absorbs the PSUM drain wait at the psA->psT transition, and
phase C reuses the still-open psT pool (psF=psT) so PSUM bank history is
already PE-observed when the feature transposes start.
"""

import sys

sys.path.insert(0, "/opt/trn_rl_repo")

import os

import numpy as np

import concourse.bass as bass
import concourse.bacc as bacc
import concourse.mybir as mybir
import concourse.tile as tile
from concourse.bass_utils import run_bass_kernel_spmd

F32 = mybir.dt.float32
BF = mybir.dt.bfloat16
I16 = mybir.dt.int16
AF = mybir.ActivationFunctionType
ALU = mybir.AluOpType

NCORES = 8
VOCAB, HID, R, B, S = 32000, 4096, 64, 50000, 16
VSH = VOCAB // NCORES          # 4000 real vocab rows per core
VSHP = 4096                    # padded vocab rows per core (32 x 128)
VOCABP = VSHP * NCORES         # 32768 padded vocab
KC = HID // 128                # 32 contraction chunks
BSH = B // NCORES              # 6250 rows per core
NT = 49                        # row tiles of 128 (6272 padded rows)
BPAD = NT * 128                # 6272
E2 = 2 * R                     # 128: packed table row [val(64)|sq(64)]
CH_T = 2                       # row-tiles per gather chunk
CHUNKS = [(i, min(CH_T, NT - i)) for i in range(0, NT, CH_T)]  # (tile0, ntiles)


def _phase_c(nc, tc, psT, rtab2, idx_sb, aux_sb, wret_sb, biasr_sb, identb_sb,
             wdt_sb, ostage, eps_sb):
    with (
        tc.tile_pool(name="g", bufs=8) as gpool,
        tc.tile_pool(name="tr", bufs=2) as tpool,
        tc.tile_pool(name="f", bufs=2) as fpool,
        tc.tile_pool(name="psG", bufs=2, space="PSUM") as psG,
    ):
        psF = psT  # reuse the open pool: bank history already PE-observed

        def finish(t0, ntile, gps):
            # res = G0 + G1*scale + G2*iscale + bias, then L2 normalize.
            # Deferred one chunk so the DVE queue never stalls on the PE/ACT
            # round-trip that produces gps.
            res = fpool.tile([128, CH_T, R], F32, tag="res")
            r2 = fpool.tile([128, CH_T, R], F32, tag="r2")
            for tt in range(ntile):
                t = t0 + tt
                nc.vector.tensor_scalar_mul(
                    res[:, tt, :], gps[:, tt, R : 2 * R],
                    aux_sb[:, NT + t : NT + t + 1])
                nc.vector.tensor_scalar_mul(
                    r2[:, tt, :], gps[:, tt, 2 * R : 3 * R],
                    aux_sb[:, 2 * NT + t : 2 * NT + t + 1])
            nc.vector.tensor_add(
                res[:, :ntile], res[:, :ntile], gps[:, :ntile, 0:R])
            nc.vector.tensor_add(res[:, :ntile], res[:, :ntile], r2[:, :ntile])
            nc.vector.tensor_tensor(
                ostage[:, t0 : t0 + ntile, :], res[:, :ntile],
                biasr_sb[:, None, :].broadcast_to([128, ntile, R]), ALU.add)
            # L2 norm (DVE-local until the one sqrt)
            sqr = fpool.tile([128, CH_T, R], F32, tag="sqr")
            nc.vector.tensor_mul(
                sqr[:, :ntile], ostage[:, t0 : t0 + ntile, :],
                ostage[:, t0 : t0 + ntile, :])
            ss = fpool.tile([128, CH_T], F32, tag="ss")
            nc.vector.tensor_reduce(
                ss[:, :ntile], sqr[:, :ntile], mybir.AxisListType.X, ALU.add)
            iss = fpool.tile([128, CH_T], F32, tag="iss")
            nc.vector.reciprocal(iss[:, :ntile], ss[:, :ntile])
            rin = fpool.tile([128, CH_T], F32, tag="rin")
            nc.scalar.sqrt(rin[:, :ntile], iss[:, :ntile])
            nc.vector.tensor_mul(
                ostage[:, t0 : t0 + ntile, :], ostage[:, t0 : t0 + ntile, :],
                rin[:, :ntile][:, :, None].broadcast_to([128, ntile, R]))

        pending = None
        stage = os.environ.get("KSTAGE", "full")
        for ci, (t0, ntile) in enumerate(CHUNKS):
            nidx = ntile * 2048
            nslot = ntile * 16
            g = gpool.tile([128, CH_T * 16, E2], BF, tag="g")
            if stage in ("gather", "full"):
                nc.gpsimd.dma_gather(
                    g[:, :nslot, :],
                    rtab2[:],
                    idx_sb[:, t0 * 128 : t0 * 128 + nidx // 16],
                    nidx,
                    nidx,
                    E2,
                    single_packet=False,
                    queue_num=ci % 4,
                )
            if stage != "full":
                continue

            g4 = g[:, :nslot, :].rearrange("p (t s) e -> p t s e", s=16)

            # -- bf16 token pooling: one halving TT then one strided reduce
            #    (fewer DVE ops -> fewer same-engine sem waits) --
            # add path runs on the full packed rows: sums values AND squares
            a1 = tpool.tile([128, CH_T, 8, E2], BF, tag="a1")
            nc.vector.tensor_tensor(
                a1[:, :ntile], g4[:, :, 0:8, :], g4[:, :, 8:16, :], ALU.add)
            addf = tpool.tile([128, CH_T, E2], F32, tag="addf")
            nc.vector.tensor_reduce(
                addf[:, :ntile],
                a1[:, :ntile].rearrange("p t s e -> p t e s"),
                mybir.AxisListType.X, ALU.add)

            featc = fpool.tile([128, CH_T, 4 * R], BF, tag="featc")

            def mmtree(op, dst_lo):
                b1 = tpool.tile([128, CH_T, 8, R], BF, tag=f"b1{dst_lo}")
                nc.vector.tensor_tensor(
                    b1[:, :ntile], g4[:, :, 0:8, 0:R], g4[:, :, 8:16, 0:R], op)
                nc.vector.tensor_reduce(
                    featc[:, :ntile, dst_lo : dst_lo + R],
                    b1[:, :ntile].rearrange("p t s e -> p t e s"),
                    mybir.AxisListType.X, op)

            mmtree(ALU.max, R)       # max -> featc[:, :, 64:128]
            mmtree(ALU.min, 2 * R)   # min -> featc[:, :, 128:192]

            # -- finals: DVE-local chains, per-tile scalars via tensor_scalar --
            sqm = fpool.tile([128, CH_T, R], F32, tag="sqm")
            for tt in range(ntile):
                t = t0 + tt
                invl = aux_sb[:, t : t + 1]
                # mean straight into featc (bf16); sq-mean staged fp32
                nc.vector.tensor_scalar_mul(
                    featc[:, tt, 0:R], addf[:, tt, 0:R], invl)
                nc.vector.tensor_scalar_mul(
                    sqm[:, tt, :], addf[:, tt, R:E2], invl)
            msq = fpool.tile([128, CH_T, R], F32, tag="msq")
            nc.vector.tensor_mul(
                msq[:, :ntile], featc[:, :ntile, 0:R], featc[:, :ntile, 0:R])
            nc.vector.tensor_tensor(
                sqm[:, :ntile], sqm[:, :ntile], msq[:, :ntile], ALU.subtract)
            # std = sqrt(var + 1e-6)  (bias folds the reference clip)
            nc.scalar.activation(
                featc[:, :ntile, 3 * R : 4 * R], sqm[:, :ntile], AF.Sqrt,
                bias=eps_sb[:, 0:1])

            # previous chunk's combine/normalize: its gps is long since done
            if pending is not None:
                finish(*pending)

            # FC per tile: G_k = features @ W_k.T via PE-transposed features
            gps = psG.tile([128, CH_T, 3 * R], F32, tag="gp", name=f"gp_{t0}")
            for tt in range(ntile):
                fts = []
                for kc in range(2):
                    ftp = psF.tile([128, 128], BF, tag="ftp")
                    nc.tensor.transpose(
                        ftp[:], featc[:, tt, kc * 128 : (kc + 1) * 128],
                        identb_sb[:],
                    )
                    ft = fpool.tile([128, 128], BF, tag=f"fts{kc}")
                    nc.scalar.activation(ft[:], ftp[:], AF.Copy)
                    fts.append(ft)
                # complete each G_k's accumulation group before the next:
                # start=True invalidates the whole bank's has_written bits,
                # so interleaved groups within one bank lose their partials
                for k in range(3):
                    for kc in range(2):
                        nc.tensor.matmul(
                            gps[:, tt, k * R : (k + 1) * R],
                            fts[kc][:],
                            wret_sb[:, kc, k * R : (k + 1) * R],
                            start=(kc == 0),
                            stop=(kc == 1),
                        )

            pending = (t0, ntile, gps)

        if stage == "full" and pending is not None:
            finish(*pending)


def build_kernel():
    nc = bacc.Bacc(
        "TRN2",
        target_bir_lowering=False,
        debug=False,
        num_devices=NCORES,
        num_swdge_queues=4,
    )
    embt = nc.declare_dram_parameter("embt", [HID, VSHP], F32, isOutput=False)
    wdt = nc.declare_dram_parameter("wdt", [HID, R], F32, isOutput=False)
    idx = nc.declare_dram_parameter("idx", [128, BPAD], I16, isOutput=False)
    aux = nc.declare_dram_parameter("aux", [128, 3 * NT], F32, isOutput=False)
    wret = nc.declare_dram_parameter("wret", [2, 128, 3 * R], BF, isOutput=False)
    biasr = nc.declare_dram_parameter("biasr", [128, R], F32, isOutput=False)
    ident = nc.declare_dram_parameter("ident", [128, 128], F32, isOutput=False)
    out = nc.declare_dram_parameter("out", [BPAD, R], F32, isOutput=True)

    with tile.TileContext(nc) as tc:
        with (
            tc.tile_pool(name="dram", bufs=1, space="DRAM") as dpool,
            tc.tile_pool(name="const", bufs=1) as cpool,
        ):
            rloc2 = dpool.tile([VSHP, E2], BF)
            rtab2 = dpool.tile([VOCABP, E2], BF)

            wdt_sb = cpool.tile([128, KC, R], F32)
            nc.sync.dma_start(wdt_sb[:], wdt.rearrange("(k p) n -> p k n", p=128))
            idx_sb = cpool.tile([128, BPAD], I16)
            nc.sync.dma_start(idx_sb[:], idx[:])
            aux_sb = cpool.tile([128, 3 * NT], F32)
            nc.sync.dma_start(aux_sb[:], aux[:])
            wret_raw = cpool.tile([128, 2, 3 * R], BF)
            nc.sync.dma_start(wret_raw[:], wret.rearrange("c p n -> p c n"))
            wret_sb = cpool.tile([128, 2, 3 * R], BF)
            nc.vector.tensor_copy(wret_sb[:], wret_raw[:])
            biasr_sb = cpool.tile([128, R], F32)
            nc.sync.dma_start(biasr_sb[:], biasr[:])
            ident_sb = cpool.tile([128, 128], F32)
            nc.sync.dma_start(ident_sb[:], ident[:])
            ostage = cpool.tile([128, NT, R], F32)

            # identity staged through DVE so PE transposes dep on DVE sem only
            ident2_sb = cpool.tile([128, 128], F32)
            nc.vector.tensor_copy(ident2_sb[:], ident_sb[:])
            identb_sb = cpool.tile([128, 128], BF)
            nc.vector.tensor_copy(identb_sb[:], ident_sb[:])
            eps_sb = cpool.tile([128, 1], F32)
            nc.gpsimd.memset(eps_sb[:], 1e-6)

            # ---- Phase A: RtabT = W_downT.T @ embT ----
            # KREPS>1 repeats the whole pipeline for launch-overhead-free
            # wall-clock measurement ((T(N)-T(1))/(N-1) = per-rep time).
            for _rep in range(int(os.environ.get("KREPS", "1"))):
              with (
                  tc.tile_pool(name="emb", bufs=3) as epool,
                  tc.tile_pool(name="stageA", bufs=1) as apool,
              ):
                  rtabT_sb = apool.tile([64, VSHP], F32)
                  with tc.tile_pool(name="psA", bufs=1, space="PSUM") as psA:
                      rtabT_ps = psA.tile([64, VSHP], F32)
                      # gate: junk matmul reading only wdt_sb -> absorbs the wdt
                      # DMA-lane wait so real matmuls carry just their ech lane
                      nc.tensor.matmul(
                          rtabT_ps[:, VSHP - 64 : VSHP - 32],
                          wdt_sb[:, 0, :],
                          wdt_sb[:, 0, 0:32],
                          start=True,
                          stop=True,
                          skip_group_check=True,
                      )
                      for k in range(KC):
                          ech = epool.tile([128, VSHP], F32, tag="ech")
                          nc.gpsimd.dma_start(ech[:], embt[k * 128 : (k + 1) * 128, :])
                          for vb in range(VSHP // 512):
                              c0 = vb * 512
                              c1 = min((vb + 1) * 512, VSHP - 64)
                              nc.tensor.matmul(
                                  rtabT_ps[:, c0:c1],
                                  wdt_sb[:, k, :],
                                  ech[:, c0:c1],
                                  start=(k == 0),
                                  stop=(k == KC - 1),
                              )
                      # absorber: junk matmul into the other pad half; its only
                      # wait is the PSUM drain (PE self-sem), freeing later
                      # matmuls from carrying it (Matmult = 1 wait max)
                      nc.tensor.matmul(
                          rtabT_ps[:, VSHP - 32 : VSHP],
                          wdt_sb[:, 0, :],
                          wdt_sb[:, 0, 32:64],
                          start=True,
                          stop=True,
                          skip_group_check=True,
                      )
                      nc.vector.tensor_copy(rtabT_sb[:], rtabT_ps[:])

                  # bf16 table slice, rows packed [val | val^2]
                  rloc2_sb = apool.tile([128, VSHP // 128, E2], BF)
                  with tc.tile_pool(name="psT", bufs=2, space="PSUM") as psT:
                      # dummy junk matmul: carries the psA->psT PSUM drain wait
                      dtp = psT.tile([64, 64], F32, tag="tp")
                      nc.tensor.matmul(
                          dtp[:], wdt_sb[:, 0, :], wdt_sb[:, 0, :],
                          start=True, stop=True,
                      )
                      nc.vector.tensor_copy(ostage[0:64, NT - 1, :], dtp[:])
                      for v in range(VSHP // 128):
                          tp = psT.tile([128, 64], F32, tag="tp")
                          nc.tensor.transpose(
                              tp[:],
                              rtabT_sb[:, v * 128 : (v + 1) * 128],
                              ident2_sb[:64, :64],
                          )
                          nc.vector.tensor_copy(rloc2_sb[:, v, 0:R], tp[:])
                          nc.scalar.square(rloc2_sb[:, v, R:E2], tp[:])
                      nc.sync.dma_start(
                          rloc2.rearrange("(v p) n -> p v n", p=128), rloc2_sb[:]
                      )

                      # ---- Phase B: AllGather rloc2 -> rtab2 ----
                      nc.gpsimd.collective_compute(
                          "AllGather",
                          ALU.bypass,
                          replica_groups=[list(range(NCORES))],
                          ins=[rloc2.opt()],
                          outs=[rtab2.opt()],
                      )

                      # ---- Phase C: gather + pool + FC ----
                      _phase_c(nc, tc, psT, rtab2, idx_sb, aux_sb, wret_sb,
                               biasr_sb, identb_sb, wdt_sb, ostage, eps_sb)

                      nc.sync.dma_start(
                          out.rearrange("(t p) n -> p t n", p=128), ostage[:]
                      )

    # Bacc's compile pipeline handles wait-limit lowering
    # (move_matmul_waits_to_ldweights, event semaphores, regalloc, ...)
    nc.compile()
    return nc


_NC_CACHE = {}


def _get_nc():
    key = (os.environ.get("KREPS", "1"), os.environ.get("KSTAGE", "full"))
    if key not in _NC_CACHE:
        _NC_CACHE[key] = build_kernel()
    return _NC_CACHE[key]


def _prepare(text_embeddings, kgl2token, W_down, W_re, b_re):
    import ml_dtypes

    emb = np.ascontiguousarray(np.asarray(text_embeddings, dtype=np.float32))
    ids = np.asarray(kgl2token)
    wd = np.asarray(W_down, dtype=np.float32)
    wr = np.asarray(W_re, dtype=np.float32)
    br = np.asarray(b_re, dtype=np.float32)

    # host-side scalars: lengths and scale factors (global mean over all rows)
    lengths = (ids > 0).sum(axis=1).astype(np.float32)  # [B]
    scale = np.log(lengths + 0.0)
    scale = scale / (scale.mean() + 1e-10)
    iscale = 1.0 / np.clip(scale, 0.01, None)
    invl = (1.0 / (lengths + 1e-10)).astype(np.float32)

    # remap ids into padded vocab layout
    ids64 = ids.astype(np.int64)
    rid = (ids64 // VSH) * VSHP + (ids64 % VSH)  # [B, S] < 32768

    wdt = np.ascontiguousarray(wd.T)  # [4096, 64]

    # W_re: result index = feat*3 + k  ->  W_k = W_re[:, k::3]  [64, 256]
    wret = np.zeros((2, 128, 3 * R), dtype=np.float32)
    for k in range(3):
        wkT = np.ascontiguousarray(wr[:, k::3].T)  # [256, 64]
        for kc in range(2):
            wret[kc, :, k * R : (k + 1) * R] = wkT[kc * 128 : (kc + 1) * 128, :]
    wret = wret.astype(ml_dtypes.bfloat16)
    biasr = np.tile(br[None, :], (128, 1)).astype(np.float32)
    identm = np.eye(128, dtype=np.float32)

    in_maps = []
    for c in range(NCORES):
        embt = np.zeros((HID, VSHP), dtype=np.float32)
        embt[:, :VSH] = emb[c * VSH : (c + 1) * VSH, :].T
        # per-core padded rows
        rid_c = np.zeros((BPAD, S), dtype=np.int64)
        rid_c[:BSH] = rid[c * BSH : (c + 1) * BSH]
        # gather order: j = t*2048 + s*128 + r
        L = rid_c.reshape(NT, 128, S).transpose(0, 2, 1).reshape(-1)  # [BPAD*S]
        idx16 = L.reshape(-1, 16).T.astype(np.int16)  # [16, BPAD]
        idxsb = np.ascontiguousarray(np.tile(idx16, (8, 1)))  # [128, BPAD]

        auxc = np.zeros((128, 3 * NT), dtype=np.float32)
        for name_i, v in enumerate((invl, scale, iscale)):
            vc = np.ones(BPAD, dtype=np.float32)
            vc[:BSH] = v[c * BSH : (c + 1) * BSH]
            auxc[:, name_i * NT : (name_i + 1) * NT] = vc.reshape(NT, 128).T
        in_maps.append(
            dict(embt=embt, wdt=wdt, idx=idxsb, aux=auxc, wret=wret,
                 biasr=biasr, ident=identm)
        )
    return in_maps, lengths, scale, iscale, invl


def _patch_rows(result, text_embeddings, kgl2token, W_down, W_re, b_re,
                scale_all, iscale_all, invl_all):
    """Recompute rows containing any id==0 token exactly (host, numpy)."""
    ids = np.asarray(kgl2token)
    bad = np.nonzero((ids <= 0).any(axis=1))[0]
    if len(bad) == 0:
        return result
    emb = np.asarray(text_embeddings, dtype=np.float32)
    wd = np.asarray(W_down, dtype=np.float32)
    wr = np.asarray(W_re, dtype=np.float32)
    br = np.asarray(b_re, dtype=np.float32)
    for r in bad:
        tok_ids = ids[r].astype(np.int64)
        tok = emb[tok_ids] @ wd.T  # [S, R]
        mask = (tok_ids > 0).astype(np.float32)[:, None]
        length = mask.sum()
        masked = tok * mask
        mean = masked.sum(axis=0) / (length + 1e-10)
        sq_mean = (tok * tok * mask).sum(axis=0) / (length + 1e-10)
        mx = (masked + (1.0 - mask) * (-1e10)).max(axis=0)
        mn = (masked + (1.0 - mask) * (1e10)).min(axis=0)
        std = np.sqrt(np.clip(sq_mean - mean * mean, 1e-6, None))
        features = np.concatenate([mean, mx, mn, std])  # [256]
        scales = np.array([1.0, scale_all[r], iscale_all[r]], dtype=np.float32)
        flat = (features[:, None] * scales[None, :]).reshape(-1)  # [768]
        res = flat @ wr.T + br
        nrm = np.linalg.norm(res)
        result[r] = res / max(nrm, 1e-12)
    return result


def kernel(text_embeddings, kgl2token, W_down, W_re, b_re, _trace=False):
    nc = _get_nc()
    in_maps, lengths, scale, iscale, invl = _prepare(
        text_embeddings, kgl2token, W_down, W_re, b_re
    )
    r = run_bass_kernel_spmd(nc, in_maps, core_ids=list(range(NCORES)), trace=_trace)
    outs = [r.results[c]["out"][:BSH] for c in range(NCORES)]
    result = np.concatenate(outs, axis=0).astype(np.float32)
    result = _patch_rows(
        result, text_embeddings, kgl2token, W_down, W_re, b_re, scale, iscale, invl
    )
    if _trace:
        return result, r
    return result



# revision 6
# speedup vs baseline: 2.1019x; 2.1019x over previous
"""BasePNARetriever Trainium2 kernel (8 NeuronCores, SPMD).

Strategy (v3):
  - Phase A (vocab-sharded down-projection) in bf16: each core streams a
    [4096, 4096] bf16 slice of text_embeddings via HWDGE (nc.sync) and
    computes RtabT[64, 4096] on PE (bf16 matmul, fp32 PSUM accumulate).
    PE-transposes back to row-major; ACT emits the bf16 table slice with
    rows packed [val(64) | val^2(64)] (256B). AllGather (Shared-addr
    output) builds the full rtab2[32768, 128] bf16 in DRAM.
  - Phase C gather desc-gen is THE wall (~3.5-8 ns/descriptor, serial on
    the GpSimd engine; 100352 descriptors/core). v3 therefore:
      * keeps GpSimd 100% dedicated to desc-gen from t~5us using
        prepare_only dma_gather preps (descriptor generation has no data
        dependency on rtab2 - only on idx_sb), with trigger_dma firing
        each chunk's DMA once the AllGather has landed (Tile defers the
        rtab2 RAW edge to the trigger automatically);
      * moves the emb streaming off SWDGE to HWDGE so emb loads never
        queue behind desc-gen on the GpSimd engine;
      * keeps every DVE op in phase C 2-port-free (tensor_tensor /
        tensor_reduce only - never tensor_scalar/copy/cast) because DVE
        2-port perf-mode ops and Q7 descriptor writes hard-block each
        other on the shared SBUF port pair; all scalar-scale/copy/square
        work runs on ACT (never contends) via activation(scale=...).
  - Pooling per 2-tile chunk: bf16 TT halving trees (sum via 2 halvings +
    f32 strided reduce; max/min via 4 halvings), mean/sq-mean/std on ACT,
    FC on PE via PE-transposed bf16 features, L2-normalize with ACT
    Square+accum_out for the row sum of squares.
  - Host precomputes lengths/log-scales and patches the rare rows
    containing id==0 tokens (~25 rows in 50000).
"""

import sys

sys.path.insert(0, "/opt/trn_rl_repo")

import os

import numpy as np

import concourse.bass as bass
import concourse.bacc as bacc
import concourse.mybir as mybir
import concourse.tile as tile
from concourse.bass_utils import run_bass_kernel_spmd

F32 = mybir.dt.float32
BF = mybir.dt.bfloat16
I16 = mybir.dt.int16
AF = mybir.ActivationFunctionType
ALU = mybir.AluOpType

NCORES = 8
VOCAB, HID, R, B, S = 32000, 4096, 64, 50000, 16
VSH = VOCAB // NCORES          # 4000 real vocab rows per core
VSHP = 4096                    # padded vocab rows per core (32 x 128)
VOCABP = VSHP * NCORES         # 32768 padded vocab
KC = HID // 128                # 32 contraction chunks
BSH = B // NCORES              # 6250 rows per core
NT = 49                        # row tiles of 128 (6272 padded rows)
BPAD = NT * 128                # 6272
E2 = 2 * R                     # 128: packed table row [val(64)|sq(64)]
CH_T = 2                       # row-tiles per gather chunk
CHUNKS = [(i, min(CH_T, NT - i)) for i in range(0, NT, CH_T)]  # (tile0, ntiles)
NQ = 4                         # SWDGE queues
AG_AT = 8                      # emit the AllGather after this many preps
TRIG_AT = 12                   # first triggers after this many preps


def _phase_c(nc, tc, psT, rtab2, idx_sb, aux_sb, wret_sb, biasr_sb, identb_sb,
             ostage, eps_sb, emit_ag):
    stage = os.environ.get("KSTAGE", "full")
    prep_mode = os.environ.get("KPREP", "1") == "1"
    nbuf = int(os.environ.get("KBUFS", "11"))
    with (
        tc.tile_pool(name="g", bufs=nbuf) as gpool,
        tc.tile_pool(name="tr", bufs=2) as tpool,
        tc.tile_pool(name="f", bufs=2) as fpool,
        tc.tile_pool(name="psG", bufs=2, space="PSUM") as psG,
    ):
        psF = psT  # reuse the open pool: bank history already PE-observed
        dma_sems = (
            [nc.alloc_semaphore(f"gsem{q}") for q in range(NQ)] if prep_mode else None
        )
        gtiles = {}
        state = {"pending": None}

        def emit_prep(ci):
            t0, ntile = CHUNKS[ci]
            nidx = ntile * 2048
            nslot = ntile * 16
            g = gpool.tile([128, CH_T * 16, E2], BF, tag="g")
            q = ci % NQ
            if stage in ("gather", "full"):
                kw = {}
                if prep_mode:
                    kw = dict(prepare_only=True, sem=dma_sems[q])
                nc.gpsimd.dma_gather(
                    g[:, :nslot, :],
                    rtab2[:],
                    idx_sb[:, t0 * 128 : t0 * 128 + nidx // 16],
                    nidx,
                    nidx,
                    E2,
                    single_packet=False,
                    queue_num=q,
                    **kw,
                )
            gtiles[ci] = g

        def emit_trigger(ci):
            if prep_mode and stage in ("gather", "full"):
                nc.gpsimd.trigger_dma(count=None, queue_num=ci % NQ)

        def finish(t0, ntile, gps):
            # res = G0 + G1*scale + G2*iscale + bias, then L2 normalize.
            # Deferred one chunk so the DVE queue never stalls on the PE/ACT
            # round-trip that produces gps.
            res = fpool.tile([128, CH_T, R], F32, tag="res")
            r2 = fpool.tile([128, CH_T, R], F32, tag="r2")
            for tt in range(ntile):
                t = t0 + tt
                nc.scalar.activation(
                    res[:, tt, :], gps[:, tt, R : 2 * R], AF.Copy,
                    scale=aux_sb[:, NT + t : NT + t + 1])
                nc.scalar.activation(
                    r2[:, tt, :], gps[:, tt, 2 * R : 3 * R], AF.Copy,
                    scale=aux_sb[:, 2 * NT + t : 2 * NT + t + 1])
            nc.vector.tensor_tensor(
                res[:, :ntile], res[:, :ntile], gps[:, :ntile, 0:R], ALU.add)
            nc.vector.tensor_add(res[:, :ntile], res[:, :ntile], r2[:, :ntile])
            nc.vector.tensor_tensor(
                ostage[:, t0 : t0 + ntile, :], res[:, :ntile],
                biasr_sb[:, None, :].broadcast_to([128, ntile, R]), ALU.add)
            # L2 norm: ACT Square + accum_out gives the row sum of squares
            sqr = fpool.tile([128, CH_T, R], F32, tag="sqr")
            ss = fpool.tile([128, CH_T], F32, tag="ss")
            for tt in range(ntile):
                nc.scalar.activation(
                    sqr[:, tt, :], ostage[:, t0 + tt, :], AF.Square,
                    accum_out=ss[:, tt : tt + 1])
            iss = fpool.tile([128, CH_T], F32, tag="iss")
            nc.vector.reciprocal(iss[:, :ntile], ss[:, :ntile])
            rin = fpool.tile([128, CH_T], F32, tag="rin")
            nc.scalar.sqrt(rin[:, :ntile], iss[:, :ntile])
            nc.vector.tensor_mul(
                ostage[:, t0 : t0 + ntile, :], ostage[:, t0 : t0 + ntile, :],
                rin[:, :ntile][:, :, None].broadcast_to([128, ntile, R]))

        def emit_pool(ci):
            if stage != "full":
                return
            t0, ntile = CHUNKS[ci]
            g = gtiles.pop(ci)
            nslot = ntile * 16
            g4 = g[:, :nslot, :].rearrange("p (t s) e -> p t s e", s=16)

            # SUM over the packed [val|sq] rows: 2 bf16 halvings (16->4)
            # then one f32 strided reduce (4->1).
            a1 = tpool.tile([128, CH_T, 8, E2], BF, tag="a1")
            nc.vector.tensor_tensor(
                a1[:, :ntile], g4[:, :, 0:8, :], g4[:, :, 8:16, :], ALU.add)
            a2 = tpool.tile([128, CH_T, 4, E2], BF, tag="a2")
            nc.vector.tensor_tensor(
                a2[:, :ntile], a1[:, :ntile, 0:4, :], a1[:, :ntile, 4:8, :],
                ALU.add)
            addf = tpool.tile([128, CH_T, E2], F32, tag="addf")
            nc.vector.tensor_reduce(
                addf[:, :ntile],
                a2[:, :ntile].rearrange("p t s e -> p t e s"),
                mybir.AxisListType.X, ALU.add)

            featc = fpool.tile([128, CH_T, 4 * R], BF, tag="featc")

            # MAX/MIN: 4 bf16 TT halvings over the val halves
            def mmtree(op, dst_lo, tag):
                h1 = tpool.tile([128, CH_T, 8, R], BF, tag=tag + "1")
                nc.vector.tensor_tensor(
                    h1[:, :ntile], g4[:, :, 0:8, 0:R], g4[:, :, 8:16, 0:R], op)
                h2 = tpool.tile([128, CH_T, 4, R], BF, tag=tag + "2")
                nc.vector.tensor_tensor(
                    h2[:, :ntile], h1[:, :ntile, 0:4, :], h1[:, :ntile, 4:8, :],
                    op)
                h3 = tpool.tile([128, CH_T, 2, R], BF, tag=tag + "3")
                nc.vector.tensor_tensor(
                    h3[:, :ntile], h2[:, :ntile, 0:2, :], h2[:, :ntile, 2:4, :],
                    op)
                nc.vector.tensor_tensor(
                    featc[:, :ntile, dst_lo : dst_lo + R],
                    h3[:, :ntile, 0, :], h3[:, :ntile, 1, :], op)

            mmtree(ALU.max, R, "mx")       # max -> featc[:, :, 64:128]
            mmtree(ALU.min, 2 * R, "mn")   # min -> featc[:, :, 128:192]

            # mean / sq-mean on ACT (per-tile 1/len scale); std via sqrt+eps
            sqm = fpool.tile([128, CH_T, R], F32, tag="sqm")
            for tt in range(ntile):
                t = t0 + tt
                invl = aux_sb[:, t : t + 1]
                nc.scalar.activation(
                    featc[:, tt, 0:R], addf[:, tt, 0:R], AF.Copy, scale=invl)
                nc.scalar.activation(
                    sqm[:, tt, :], addf[:, tt, R:E2], AF.Copy, scale=invl)
            msq = fpool.tile([128, CH_T, R], F32, tag="msq")
            nc.scalar.activation(
                msq[:, :ntile], featc[:, :ntile, 0:R], AF.Square)
            nc.vector.tensor_tensor(
                sqm[:, :ntile], sqm[:, :ntile], msq[:, :ntile], ALU.subtract)
            nc.scalar.activation(
                featc[:, :ntile, 3 * R : 4 * R], sqm[:, :ntile], AF.Sqrt,
                bias=eps_sb[:, 0:1])

            # previous chunk's combine/normalize
            if state["pending"] is not None:
                finish(*state["pending"])

            # FC per tile: G_k = features @ W_k.T via PE-transposed features
            gps = psG.tile([128, CH_T, 3 * R], F32, tag="gp", name=f"gp_{t0}")
            for tt in range(ntile):
                fts = []
                for kc in range(2):
                    ftp = psF.tile([128, 128], BF, tag="ftp")
                    nc.tensor.transpose(
                        ftp[:], featc[:, tt, kc * 128 : (kc + 1) * 128],
                        identb_sb[:],
                    )
                    ft = fpool.tile([128, 128], BF, tag=f"fts{kc}")
                    nc.scalar.activation(ft[:], ftp[:], AF.Copy)
                    fts.append(ft)
                # complete each G_k's accumulation group before the next
                for k in range(3):
                    for kc in range(2):
                        nc.tensor.matmul(
                            gps[:, tt, k * R : (k + 1) * R],
                            fts[kc][:],
                            wret_sb[:, kc, k * R : (k + 1) * R],
                            start=(kc == 0),
                            stop=(kc == 1),
                        )

            state["pending"] = (t0, ntile, gps)

        # --- emission schedule ---
        if not prep_mode:
            # non-prepared gathers read rtab2 at desc-gen time: the AG must
            # precede the first gather in the gpsimd stream
            emit_ag()
            for ci in range(len(CHUNKS)):
                emit_prep(ci)
                emit_pool(ci)
        else:
            # gpsimd stream: p0..p8, AG, p9..p12, T0..T3, (p_k, t_k)*
            for ci in range(len(CHUNKS)):
                emit_prep(ci)
                if ci == AG_AT:
                    emit_ag()
                if ci == TRIG_AT:
                    for q in range(NQ):
                        if stage in ("gather", "full"):
                            nc.gpsimd.trigger_dma(count=None, queue_num=q)
                    for cj in range(TRIG_AT + 1):
                        emit_pool(cj)
                elif ci > TRIG_AT:
                    emit_trigger(ci)
                    emit_pool(ci)

        if stage == "full" and state["pending"] is not None:
            finish(*state["pending"])


def build_kernel():
    nc = bacc.Bacc(
        "TRN2",
        target_bir_lowering=False,
        debug=False,
        num_devices=NCORES,
        num_swdge_queues=NQ,
    )
    embt = nc.declare_dram_parameter("embt", [HID, VSHP], BF, isOutput=False)
    wdt = nc.declare_dram_parameter("wdt", [HID, R], BF, isOutput=False)
    idx = nc.declare_dram_parameter("idx", [128, BPAD], I16, isOutput=False)
    aux = nc.declare_dram_parameter("aux", [128, 3 * NT], F32, isOutput=False)
    wret = nc.declare_dram_parameter("wret", [2, 128, 3 * R], BF, isOutput=False)
    biasr = nc.declare_dram_parameter("biasr", [128, R], F32, isOutput=False)
    ident = nc.declare_dram_parameter("ident", [128, 128], F32, isOutput=False)
    out = nc.declare_dram_parameter("out", [BPAD, R], F32, isOutput=True)

    with tile.TileContext(nc) as tc:
        with (
            tc.tile_pool(name="dram", bufs=1, space="DRAM") as dpool,
            tc.tile_pool(name="const", bufs=1) as cpool,
        ):
            rloc2 = dpool.tile([VSHP, E2], BF)
            rtab2 = dpool.tile([VOCABP, E2], BF)

            wdt_sb = cpool.tile([128, KC, R], BF)
            nc.sync.dma_start(wdt_sb[:], wdt.rearrange("(k p) n -> p k n", p=128))
            idx_sb = cpool.tile([128, BPAD], I16)
            nc.sync.dma_start(idx_sb[:], idx[:])
            aux_sb = cpool.tile([128, 3 * NT], F32)
            nc.sync.dma_start(aux_sb[:], aux[:])
            wret_raw = cpool.tile([128, 2, 3 * R], BF)
            nc.sync.dma_start(wret_raw[:], wret.rearrange("c p n -> p c n"))
            wret_sb = cpool.tile([128, 2, 3 * R], BF)
            nc.scalar.activation(wret_sb[:], wret_raw[:], AF.Copy)
            biasr_sb = cpool.tile([128, R], F32)
            nc.sync.dma_start(biasr_sb[:], biasr[:])
            ident_sb = cpool.tile([128, 128], F32)
            nc.sync.dma_start(ident_sb[:], ident[:])
            ostage = cpool.tile([128, NT, R], F32)

            # identity staged through ACT so PE transposes dep on ACT sem only
            ident2_sb = cpool.tile([128, 128], F32)
            nc.scalar.activation(ident2_sb[:], ident_sb[:], AF.Copy)
            identb_sb = cpool.tile([128, 128], BF)
            nc.scalar.activation(identb_sb[:], ident_sb[:], AF.Copy)
            eps_sb = cpool.tile([128, 1], F32)
            nc.gpsimd.memset(eps_sb[:], 1e-6)

            # ---- Phase A: RtabT = W_downT.T @ embT (bf16) ----
            for _rep in range(int(os.environ.get("KREPS", "1"))):
              with (
                  tc.tile_pool(name="emb", bufs=3) as epool,
                  tc.tile_pool(name="stageA", bufs=1) as apool,
              ):
                  rtabT_sb = apool.tile([64, VSHP], F32)
                  with tc.tile_pool(name="psA", bufs=1, space="PSUM") as psA:
                      rtabT_ps = psA.tile([64, VSHP], F32)
                      # gate: junk matmul reading only wdt_sb -> absorbs the wdt
                      # DMA-lane wait so real matmuls carry just their ech lane
                      nc.tensor.matmul(
                          rtabT_ps[:, VSHP - 64 : VSHP - 32],
                          wdt_sb[:, 0, :],
                          wdt_sb[:, 0, 0:32],
                          start=True,
                          stop=True,
                          skip_group_check=True,
                      )
                      for k in range(KC):
                          ech = epool.tile([128, VSHP], BF, tag="ech")
                          nc.sync.dma_start(ech[:], embt[k * 128 : (k + 1) * 128, :])
                          for vb in range(VSHP // 512):
                              c0 = vb * 512
                              c1 = min((vb + 1) * 512, VSHP - 64)
                              nc.tensor.matmul(
                                  rtabT_ps[:, c0:c1],
                                  wdt_sb[:, k, :],
                                  ech[:, c0:c1],
                                  start=(k == 0),
                                  stop=(k == KC - 1),
                              )
                      # absorber: junk matmul into the other pad half carries
                      # the PSUM drain wait (Matmult = 1 wait max)
                      nc.tensor.matmul(
                          rtabT_ps[:, VSHP - 32 : VSHP],
                          wdt_sb[:, 0, :],
                          wdt_sb[:, 0, 32:64],
                          start=True,
                          stop=True,
                          skip_group_check=True,
                      )
                      nc.scalar.activation(rtabT_sb[:], rtabT_ps[:], AF.Copy)

                  # bf16 table slice, rows packed [val | val^2]
                  rloc2_sb = apool.tile([128, VSHP // 128, E2], BF)
                  with tc.tile_pool(name="psT", bufs=2, space="PSUM") as psT:
                      # dummy junk matmul: carries the psA->psT PSUM drain wait
                      dtp = psT.tile([64, 64], F32, tag="tp")
                      nc.tensor.matmul(
                          dtp[:], wdt_sb[:, 0, :], wdt_sb[:, 0, :],
                          start=True, stop=True,
                      )
                      nc.scalar.activation(
                          ostage[0:64, NT - 1, :], dtp[:], AF.Copy)
                      for v in range(VSHP // 128):
                          tp = psT.tile([128, 64], F32, tag="tp")
                          nc.tensor.transpose(
                              tp[:],
                              rtabT_sb[:, v * 128 : (v + 1) * 128],
                              ident2_sb[:64, :64],
                          )
                          nc.scalar.activation(
                              rloc2_sb[:, v, 0:R], tp[:], AF.Copy)
                          nc.scalar.activation(
                              rloc2_sb[:, v, R:E2], tp[:], AF.Square)
                      nc.sync.dma_start(
                          rloc2.rearrange("(v p) n -> p v n", p=128), rloc2_sb[:]
                      )

                      def emit_ag():
                          # ---- Phase B: AllGather rloc2 -> rtab2 ----
                          nc.gpsimd.collective_compute(
                              "AllGather",
                              ALU.bypass,
                              replica_groups=[list(range(NCORES))],
                              ins=[rloc2.opt()],
                              outs=[rtab2.opt()],
                          )

                      # ---- Phase C: gather + pool + FC ----
                      _phase_c(nc, tc, psT, rtab2, idx_sb, aux_sb, wret_sb,
                               biasr_sb, identb_sb, ostage, eps_sb, emit_ag)

                      nc.sync.dma_start(
                          out.rearrange("(t p) n -> p t n", p=128), ostage[:]
                      )

    nc.compile()
    return nc


_NC_CACHE = {}


def _get_nc():
    key = (
        os.environ.get("KREPS", "1"),
        os.environ.get("KSTAGE", "full"),
        os.environ.get("KPREP", "1"),
        os.environ.get("KBUFS", "11"),
    )
    if key not in _NC_CACHE:
        _NC_CACHE[key] = build_kernel()
    return _NC_CACHE[key]


def _prepare(text_embeddings, kgl2token, W_down, W_re, b_re):
    import ml_dtypes

    emb = np.asarray(text_embeddings, dtype=np.float32)
    ids = np.asarray(kgl2token)
    wd = np.asarray(W_down, dtype=np.float32)
    wr = np.asarray(W_re, dtype=np.float32)
    br = np.asarray(b_re, dtype=np.float32)

    # host-side scalars: lengths and scale factors (global mean over all rows)
    lengths = (ids > 0).sum(axis=1).astype(np.float32)  # [B]
    scale = np.log(lengths + 0.0)
    scale = scale / (scale.mean() + 1e-10)
    iscale = 1.0 / np.clip(scale, 0.01, None)
    invl = (1.0 / (lengths + 1e-10)).astype(np.float32)

    # remap ids into padded vocab layout
    ids64 = ids.astype(np.int64)
    rid = (ids64 // VSH) * VSHP + (ids64 % VSH)  # [B, S] < 32768

    wdt = np.ascontiguousarray(wd.T).astype(ml_dtypes.bfloat16)  # [4096, 64]

    # W_re: result index = feat*3 + k  ->  W_k = W_re[:, k::3]  [64, 256]
    wret = np.zeros((2, 128, 3 * R), dtype=np.float32)
    for k in range(3):
        wkT = np.ascontiguousarray(wr[:, k::3].T)  # [256, 64]
        for kc in range(2):
            wret[kc, :, k * R : (k + 1) * R] = wkT[kc * 128 : (kc + 1) * 128, :]
    wret = wret.astype(ml_dtypes.bfloat16)
    biasr = np.tile(br[None, :], (128, 1)).astype(np.float32)
    identm = np.eye(128, dtype=np.float32)

    in_maps = []
    for c in range(NCORES):
        embt = np.zeros((HID, VSHP), dtype=ml_dtypes.bfloat16)
        embt[:, :VSH] = emb[c * VSH : (c + 1) * VSH, :].T.astype(ml_dtypes.bfloat16)
        # per-core padded rows
        rid_c = np.zeros((BPAD, S), dtype=np.int64)
        rid_c[:BSH] = rid[c * BSH : (c + 1) * BSH]
        # gather order: j = t*2048 + s*128 + r
        L = rid_c.reshape(NT, 128, S).transpose(0, 2, 1).reshape(-1)  # [BPAD*S]
        idx16 = L.reshape(-1, 16).T.astype(np.int16)  # [16, BPAD]
        idxsb = np.ascontiguousarray(np.tile(idx16, (8, 1)))  # [128, BPAD]

        auxc = np.zeros((128, 3 * NT), dtype=np.float32)
        for name_i, v in enumerate((invl, scale, iscale)):
            vc = np.ones(BPAD, dtype=np.float32)
            vc[:BSH] = v[c * BSH : (c + 1) * BSH]
            auxc[:, name_i * NT : (name_i + 1) * NT] = vc.reshape(NT, 128).T
        in_maps.append(
            dict(embt=embt, wdt=wdt, idx=idxsb, aux=auxc, wret=wret,
                 biasr=biasr, ident=identm)
        )
    return in_maps, lengths, scale, iscale, invl


def _patch_rows(result, text_embeddings, kgl2token, W_down, W_re, b_re,
                scale_all, iscale_all, invl_all):
    """Recompute rows containing any id==0 token exactly (host, numpy)."""
    ids = np.asarray(kgl2token)
    bad = np.nonzero((ids <= 0).any(axis=1))[0]
    if len(bad) == 0:
        return result
    emb = np.asarray(text_embeddings, dtype=np.float32)
    wd = np.asarray(W_down, dtype=np.float32)
    wr = np.asarray(W_re, dtype=np.float32)
    br = np.asarray(b_re, dtype=np.float32)
    for r in bad:
        tok_ids = ids[r].astype(np.int64)
        tok = emb[tok_ids] @ wd.T  # [S, R]
        mask = (tok_ids > 0).astype(np.float32)[:, None]
        length = mask.sum()
        masked = tok * mask
        mean = masked.sum(axis=0) / (length + 1e-10)
        sq_mean = (tok * tok * mask).sum(axis=0) / (length + 1e-10)
        mx = (masked + (1.0 - mask) * (-1e10)).max(axis=0)
        mn = (masked + (1.0 - mask) * (1e10)).min(axis=0)
        std = np.sqrt(np.clip(sq_mean - mean * mean, 1e-6, None))
        features = np.concatenate([mean, mx, mn, std])  # [256]
        scales = np.array([1.0, scale_all[r], iscale_all[r]], dtype=np.float32)
        flat = (features[:, None] * scales[None, :]).reshape(-1)  # [768]
        res = flat @ wr.T + br
        nrm = np.linalg.norm(res)
        result[r] = res / max(nrm, 1e-12)
    return result


def kernel(text_embeddings, kgl2token, W_down, W_re, b_re, _trace=False):
    nc = _get_nc()
    in_maps, lengths, scale, iscale, invl = _prepare(
        text_embeddings, kgl2token, W_down, W_re, b_re
    )
    r = run_bass_kernel_spmd(nc, in_maps, core_ids=list(range(NCORES)), trace=_trace)
    outs = [r.results[c]["out"][:BSH] for c in range(NCORES)]
    result = np.concatenate(outs, axis=0).astype(np.float32)
    result = _patch_rows(
        result, text_embeddings, kgl2token, W_down, W_re, b_re, scale, iscale, invl
    )
    if _trace:
        return result, r
    return result


# revision 7
# speedup vs baseline: 2.1222x; 1.0097x over previous
"""BasePNARetriever Trainium2 kernel (8 NeuronCores, SPMD).

Strategy (v3):
  - Phase A (vocab-sharded down-projection) in bf16: each core streams a
    [4096, 4096] bf16 slice of text_embeddings via HWDGE (nc.sync) and
    computes RtabT[64, 4096] on PE (bf16 matmul, fp32 PSUM accumulate).
    PE-transposes back to row-major; ACT emits the bf16 table slice with
    rows packed [val(64) | val^2(64)] (256B). AllGather (Shared-addr
    output) builds the full rtab2[32768, 128] bf16 in DRAM.
  - Phase C gather desc-gen is THE wall (~3.5-8 ns/descriptor, serial on
    the GpSimd engine; 100352 descriptors/core). v3 therefore:
      * keeps GpSimd 100% dedicated to desc-gen from t~5us using
        prepare_only dma_gather preps (descriptor generation has no data
        dependency on rtab2 - only on idx_sb), with trigger_dma firing
        each chunk's DMA once the AllGather has landed (Tile defers the
        rtab2 RAW edge to the trigger automatically);
      * moves the emb streaming off SWDGE to HWDGE so emb loads never
        queue behind desc-gen on the GpSimd engine;
      * keeps every DVE op in phase C 2-port-free (tensor_tensor /
        tensor_reduce only - never tensor_scalar/copy/cast) because DVE
        2-port perf-mode ops and Q7 descriptor writes hard-block each
        other on the shared SBUF port pair; all scalar-scale/copy/square
        work runs on ACT (never contends) via activation(scale=...).
  - Pooling per 2-tile chunk: bf16 TT halving trees (sum via 2 halvings +
    f32 strided reduce; max/min via 4 halvings), mean/sq-mean/std on ACT,
    FC on PE via PE-transposed bf16 features, L2-normalize with ACT
    Square+accum_out for the row sum of squares.
  - Host precomputes lengths/log-scales and patches the rare rows
    containing id==0 tokens (~25 rows in 50000).
"""

import sys

sys.path.insert(0, "/opt/trn_rl_repo")

import os

import numpy as np

import concourse.bass as bass
import concourse.bacc as bacc
import concourse.mybir as mybir
import concourse.tile as tile
from concourse.bass_utils import run_bass_kernel_spmd

F32 = mybir.dt.float32
BF = mybir.dt.bfloat16
I16 = mybir.dt.int16
AF = mybir.ActivationFunctionType
ALU = mybir.AluOpType

NCORES = 8
VOCAB, HID, R, B, S = 32000, 4096, 64, 50000, 16
VSH = VOCAB // NCORES          # 4000 real vocab rows per core
VSHP = 4096                    # padded vocab rows per core (32 x 128)
VOCABP = VSHP * NCORES         # 32768 padded vocab
KC = HID // 128                # 32 contraction chunks
BSH = B // NCORES              # 6250 rows per core
NT = 49                        # row tiles of 128 (6272 padded rows)
BPAD = NT * 128                # 6272
E2 = 2 * R                     # 128: packed table row [val(64)|sq(64)]
CH_T = 2                       # row-tiles per gather chunk
CHUNKS = [(i, min(CH_T, NT - i)) for i in range(0, NT, CH_T)]  # (tile0, ntiles)
NQ = 4                         # SWDGE queues
AG_AT = 8                      # emit the AllGather after this many preps
TRIG_AT = 12                   # first triggers after this many preps


def _phase_c(nc, tc, psT, rtab2, idx_sb, aux_sb, wret_sb, biasr_sb, identb_sb,
             ostage, eps_sb, emit_ag):
    stage = os.environ.get("KSTAGE", "full")
    prep_mode = os.environ.get("KPREP", "1") == "1"
    nbuf = int(os.environ.get("KBUFS", "11"))
    with (
        tc.tile_pool(name="g", bufs=nbuf) as gpool,
        tc.tile_pool(name="tr", bufs=2) as tpool,
        tc.tile_pool(name="f", bufs=2) as fpool,
        tc.tile_pool(name="psG", bufs=2, space="PSUM") as psG,
    ):
        psF = psT  # reuse the open pool: bank history already PE-observed
        dma_sems = (
            [nc.alloc_semaphore(f"gsem{q}") for q in range(NQ)] if prep_mode else None
        )
        gtiles = {}
        state = {"pending": None}

        def emit_prep(ci):
            t0, ntile = CHUNKS[ci]
            nidx = ntile * 2048
            nslot = ntile * 16
            g = gpool.tile([128, CH_T * 16, E2], BF, tag="g")
            q = ci % NQ
            if stage in ("gather", "full"):
                kw = {}
                if prep_mode:
                    kw = dict(prepare_only=True, sem=dma_sems[q])
                nc.gpsimd.dma_gather(
                    g[:, :nslot, :],
                    rtab2[:],
                    idx_sb[:, t0 * 128 : t0 * 128 + nidx // 16],
                    nidx,
                    nidx,
                    E2,
                    single_packet=False,
                    queue_num=q,
                    **kw,
                )
            gtiles[ci] = g

        def emit_trigger(ci):
            if prep_mode and stage in ("gather", "full"):
                nc.gpsimd.trigger_dma(count=None, queue_num=ci % NQ)

        def finish(t0, ntile, gps):
            # res = G0 + G1*scale + G2*iscale + bias, then L2 normalize.
            # Deferred one chunk so the DVE queue never stalls on the PE/ACT
            # round-trip that produces gps.
            res = fpool.tile([128, CH_T, R], F32, tag="res")
            r2 = fpool.tile([128, CH_T, R], F32, tag="r2")
            for tt in range(ntile):
                t = t0 + tt
                nc.scalar.activation(
                    res[:, tt, :], gps[:, tt, R : 2 * R], AF.Copy,
                    scale=aux_sb[:, NT + t : NT + t + 1])
                nc.scalar.activation(
                    r2[:, tt, :], gps[:, tt, 2 * R : 3 * R], AF.Copy,
                    scale=aux_sb[:, 2 * NT + t : 2 * NT + t + 1])
            nc.vector.tensor_tensor(
                res[:, :ntile], res[:, :ntile], gps[:, :ntile, 0:R], ALU.add)
            nc.vector.tensor_add(res[:, :ntile], res[:, :ntile], r2[:, :ntile])
            nc.vector.tensor_tensor(
                ostage[:, t0 : t0 + ntile, :], res[:, :ntile],
                biasr_sb[:, None, :].broadcast_to([128, ntile, R]), ALU.add)
            # L2 norm: ACT Square + accum_out gives the row sum of squares
            sqr = fpool.tile([128, CH_T, R], F32, tag="sqr")
            ss = fpool.tile([128, CH_T], F32, tag="ss")
            for tt in range(ntile):
                nc.scalar.activation(
                    sqr[:, tt, :], ostage[:, t0 + tt, :], AF.Square,
                    accum_out=ss[:, tt : tt + 1])
            iss = fpool.tile([128, CH_T], F32, tag="iss")
            nc.vector.reciprocal(iss[:, :ntile], ss[:, :ntile])
            rin = fpool.tile([128, CH_T], F32, tag="rin")
            nc.scalar.sqrt(rin[:, :ntile], iss[:, :ntile])
            nc.vector.tensor_mul(
                ostage[:, t0 : t0 + ntile, :], ostage[:, t0 : t0 + ntile, :],
                rin[:, :ntile][:, :, None].broadcast_to([128, ntile, R]))

        def emit_pool(ci):
            if stage != "full":
                return
            t0, ntile = CHUNKS[ci]
            g = gtiles.pop(ci)
            nslot = ntile * 16
            g4 = g[:, :nslot, :].rearrange("p (t s) e -> p t s e", s=16)

            # SUM over the packed [val|sq] rows: 2 bf16 halvings (16->4)
            # then one f32 strided reduce (4->1).
            a1 = tpool.tile([128, CH_T, 8, E2], BF, tag="a1")
            nc.vector.tensor_tensor(
                a1[:, :ntile], g4[:, :, 0:8, :], g4[:, :, 8:16, :], ALU.add)
            a2 = tpool.tile([128, CH_T, 4, E2], BF, tag="a2")
            nc.vector.tensor_tensor(
                a2[:, :ntile], a1[:, :ntile, 0:4, :], a1[:, :ntile, 4:8, :],
                ALU.add)
            addf = tpool.tile([128, CH_T, E2], F32, tag="addf")
            nc.vector.tensor_reduce(
                addf[:, :ntile],
                a2[:, :ntile].rearrange("p t s e -> p t e s"),
                mybir.AxisListType.X, ALU.add)

            featc = fpool.tile([128, CH_T, 4 * R], BF, tag="featc")

            # MAX/MIN: 4 bf16 TT halvings over the val halves
            def mmtree(op, dst_lo, tag):
                h1 = tpool.tile([128, CH_T, 8, R], BF, tag=tag + "1")
                nc.vector.tensor_tensor(
                    h1[:, :ntile], g4[:, :, 0:8, 0:R], g4[:, :, 8:16, 0:R], op)
                h2 = tpool.tile([128, CH_T, 4, R], BF, tag=tag + "2")
                nc.vector.tensor_tensor(
                    h2[:, :ntile], h1[:, :ntile, 0:4, :], h1[:, :ntile, 4:8, :],
                    op)
                h3 = tpool.tile([128, CH_T, 2, R], BF, tag=tag + "3")
                nc.vector.tensor_tensor(
                    h3[:, :ntile], h2[:, :ntile, 0:2, :], h2[:, :ntile, 2:4, :],
                    op)
                nc.vector.tensor_tensor(
                    featc[:, :ntile, dst_lo : dst_lo + R],
                    h3[:, :ntile, 0, :], h3[:, :ntile, 1, :], op)

            mmtree(ALU.max, R, "mx")       # max -> featc[:, :, 64:128]
            mmtree(ALU.min, 2 * R, "mn")   # min -> featc[:, :, 128:192]

            # mean / sq-mean on ACT (per-tile 1/len scale); std via sqrt+eps
            sqm = fpool.tile([128, CH_T, R], F32, tag="sqm")
            for tt in range(ntile):
                t = t0 + tt
                invl = aux_sb[:, t : t + 1]
                nc.scalar.activation(
                    featc[:, tt, 0:R], addf[:, tt, 0:R], AF.Copy, scale=invl)
                nc.scalar.activation(
                    sqm[:, tt, :], addf[:, tt, R:E2], AF.Copy, scale=invl)
            msq = fpool.tile([128, CH_T, R], F32, tag="msq")
            nc.scalar.activation(
                msq[:, :ntile], featc[:, :ntile, 0:R], AF.Square)
            nc.vector.tensor_tensor(
                sqm[:, :ntile], sqm[:, :ntile], msq[:, :ntile], ALU.subtract)
            nc.scalar.activation(
                featc[:, :ntile, 3 * R : 4 * R], sqm[:, :ntile], AF.Sqrt,
                bias=eps_sb[:, 0:1])

            # previous chunk's combine/normalize
            if state["pending"] is not None:
                finish(*state["pending"])

            # FC per tile: G_k = features @ W_k.T via PE-transposed features
            gps = psG.tile([128, CH_T, 3 * R], F32, tag="gp", name=f"gp_{t0}")
            for tt in range(ntile):
                fts = []
                for kc in range(2):
                    ftp = psF.tile([128, 128], BF, tag="ftp")
                    nc.tensor.transpose(
                        ftp[:], featc[:, tt, kc * 128 : (kc + 1) * 128],
                        identb_sb[:],
                    )
                    ft = fpool.tile([128, 128], BF, tag=f"fts{kc}")
                    nc.scalar.activation(ft[:], ftp[:], AF.Copy)
                    fts.append(ft)
                # complete each G_k's accumulation group before the next
                for k in range(3):
                    for kc in range(2):
                        nc.tensor.matmul(
                            gps[:, tt, k * R : (k + 1) * R],
                            fts[kc][:],
                            wret_sb[:, kc, k * R : (k + 1) * R],
                            start=(kc == 0),
                            stop=(kc == 1),
                        )

            state["pending"] = (t0, ntile, gps)

        # --- emission schedule ---
        if not prep_mode:
            # non-prepared gathers read rtab2 at desc-gen time: the AG must
            # precede the first gather in the gpsimd stream
            emit_ag()
            for ci in range(len(CHUNKS)):
                emit_prep(ci)
                emit_pool(ci)
        else:
            # gpsimd stream: p0..p8, AG, p9..p12, T0..T3, (p_k, t_k)*
            for ci in range(len(CHUNKS)):
                emit_prep(ci)
                if ci == AG_AT:
                    emit_ag()
                if ci == TRIG_AT:
                    for q in range(NQ):
                        if stage in ("gather", "full"):
                            nc.gpsimd.trigger_dma(count=None, queue_num=q)
                    for cj in range(TRIG_AT + 1):
                        emit_pool(cj)
                elif ci > TRIG_AT:
                    emit_trigger(ci)
                    emit_pool(ci)

        if stage == "full" and state["pending"] is not None:
            finish(*state["pending"])


def build_kernel():
    nc = bacc.Bacc(
        "TRN2",
        target_bir_lowering=False,
        debug=False,
        num_devices=NCORES,
        num_swdge_queues=NQ,
    )
    embt = nc.declare_dram_parameter("embt", [HID, VSHP], BF, isOutput=False)
    wdt = nc.declare_dram_parameter("wdt", [HID, R], BF, isOutput=False)
    idx = nc.declare_dram_parameter("idx", [128, BPAD], I16, isOutput=False)
    aux = nc.declare_dram_parameter("aux", [128, 3 * NT], F32, isOutput=False)
    wret = nc.declare_dram_parameter("wret", [2, 128, 3 * R], BF, isOutput=False)
    biasr = nc.declare_dram_parameter("biasr", [128, R], F32, isOutput=False)
    ident = nc.declare_dram_parameter("ident", [128, 128], F32, isOutput=False)
    out = nc.declare_dram_parameter("out", [BPAD, R], F32, isOutput=True)

    with tile.TileContext(nc) as tc:
        with (
            tc.tile_pool(name="dram", bufs=1, space="DRAM") as dpool,
            tc.tile_pool(name="const", bufs=1) as cpool,
        ):
            rloc2 = dpool.tile([VSHP, E2], BF)
            rtab2 = dpool.tile([VOCABP, E2], BF, addr_space="Shared")

            wdt_sb = cpool.tile([128, KC, R], BF)
            nc.sync.dma_start(wdt_sb[:], wdt.rearrange("(k p) n -> p k n", p=128))
            idx_sb = cpool.tile([128, BPAD], I16)
            nc.sync.dma_start(idx_sb[:], idx[:])
            aux_sb = cpool.tile([128, 3 * NT], F32)
            nc.sync.dma_start(aux_sb[:], aux[:])
            wret_raw = cpool.tile([128, 2, 3 * R], BF)
            nc.sync.dma_start(wret_raw[:], wret.rearrange("c p n -> p c n"))
            wret_sb = cpool.tile([128, 2, 3 * R], BF)
            nc.scalar.activation(wret_sb[:], wret_raw[:], AF.Copy)
            biasr_sb = cpool.tile([128, R], F32)
            nc.sync.dma_start(biasr_sb[:], biasr[:])
            ident_sb = cpool.tile([128, 128], F32)
            nc.sync.dma_start(ident_sb[:], ident[:])
            ostage = cpool.tile([128, NT, R], F32)

            # identity staged through ACT so PE transposes dep on ACT sem only
            ident2_sb = cpool.tile([128, 128], F32)
            nc.scalar.activation(ident2_sb[:], ident_sb[:], AF.Copy)
            identb_sb = cpool.tile([128, 128], BF)
            nc.scalar.activation(identb_sb[:], ident_sb[:], AF.Copy)
            eps_sb = cpool.tile([128, 1], F32)
            nc.gpsimd.memset(eps_sb[:], 1e-6)

            # ---- Phase A: RtabT = W_downT.T @ embT (bf16) ----
            for _rep in range(int(os.environ.get("KREPS", "1"))):
              with (
                  tc.tile_pool(name="emb", bufs=3) as epool,
                  tc.tile_pool(name="stageA", bufs=1) as apool,
              ):
                  rtabT_sb = apool.tile([64, VSHP], F32)
                  with tc.tile_pool(name="psA", bufs=1, space="PSUM") as psA:
                      rtabT_ps = psA.tile([64, VSHP], F32)
                      # gate: junk matmul reading only wdt_sb -> absorbs the wdt
                      # DMA-lane wait so real matmuls carry just their ech lane
                      nc.tensor.matmul(
                          rtabT_ps[:, VSHP - 64 : VSHP - 32],
                          wdt_sb[:, 0, :],
                          wdt_sb[:, 0, 0:32],
                          start=True,
                          stop=True,
                          skip_group_check=True,
                      )
                      for k in range(KC):
                          ech = epool.tile([128, VSHP], BF, tag="ech")
                          nc.sync.dma_start(ech[:], embt[k * 128 : (k + 1) * 128, :])
                          for vb in range(VSHP // 512):
                              c0 = vb * 512
                              c1 = min((vb + 1) * 512, VSHP - 64)
                              nc.tensor.matmul(
                                  rtabT_ps[:, c0:c1],
                                  wdt_sb[:, k, :],
                                  ech[:, c0:c1],
                                  start=(k == 0),
                                  stop=(k == KC - 1),
                              )
                      # absorber: junk matmul into the other pad half carries
                      # the PSUM drain wait (Matmult = 1 wait max)
                      nc.tensor.matmul(
                          rtabT_ps[:, VSHP - 32 : VSHP],
                          wdt_sb[:, 0, :],
                          wdt_sb[:, 0, 32:64],
                          start=True,
                          stop=True,
                          skip_group_check=True,
                      )
                      nc.scalar.activation(rtabT_sb[:], rtabT_ps[:], AF.Copy)

                  # bf16 table slice, rows packed [val | val^2]
                  rloc2_sb = apool.tile([128, VSHP // 128, E2], BF)
                  with tc.tile_pool(name="psT", bufs=2, space="PSUM") as psT:
                      # dummy junk matmul: carries the psA->psT PSUM drain wait
                      dtp = psT.tile([64, 64], F32, tag="tp")
                      nc.tensor.matmul(
                          dtp[:], wdt_sb[:, 0, :], wdt_sb[:, 0, :],
                          start=True, stop=True,
                      )
                      nc.scalar.activation(
                          ostage[0:64, NT - 1, :], dtp[:], AF.Copy)
                      for v in range(VSHP // 128):
                          tp = psT.tile([128, 64], F32, tag="tp")
                          nc.tensor.transpose(
                              tp[:],
                              rtabT_sb[:, v * 128 : (v + 1) * 128],
                              ident2_sb[:64, :64],
                          )
                          nc.scalar.activation(
                              rloc2_sb[:, v, 0:R], tp[:], AF.Copy)
                          nc.scalar.activation(
                              rloc2_sb[:, v, R:E2], tp[:], AF.Square)
                      nc.sync.dma_start(
                          rloc2.rearrange("(v p) n -> p v n", p=128), rloc2_sb[:]
                      )

                      def emit_ag():
                          # ---- Phase B: AllGather rloc2 -> rtab2 ----
                          nc.gpsimd.collective_compute(
                              "AllGather",
                              ALU.bypass,
                              replica_groups=[list(range(NCORES))],
                              ins=[rloc2.opt()],
                              outs=[rtab2.opt()],
                          )

                      # ---- Phase C: gather + pool + FC ----
                      _phase_c(nc, tc, psT, rtab2, idx_sb, aux_sb, wret_sb,
                               biasr_sb, identb_sb, ostage, eps_sb, emit_ag)

                      nc.sync.dma_start(
                          out.rearrange("(t p) n -> p t n", p=128), ostage[:]
                      )

    nc.compile()
    return nc


_NC_CACHE = {}


def _get_nc():
    key = (
        os.environ.get("KREPS", "1"),
        os.environ.get("KSTAGE", "full"),
        os.environ.get("KPREP", "1"),
        os.environ.get("KBUFS", "11"),
    )
    if key not in _NC_CACHE:
        _NC_CACHE[key] = build_kernel()
    return _NC_CACHE[key]


def _prepare(text_embeddings, kgl2token, W_down, W_re, b_re):
    import ml_dtypes

    emb = np.asarray(text_embeddings, dtype=np.float32)
    ids = np.asarray(kgl2token)
    wd = np.asarray(W_down, dtype=np.float32)
    wr = np.asarray(W_re, dtype=np.float32)
    br = np.asarray(b_re, dtype=np.float32)

    # host-side scalars: lengths and scale factors (global mean over all rows)
    lengths = (ids > 0).sum(axis=1).astype(np.float32)  # [B]
    scale = np.log(lengths + 0.0)
    scale = scale / (scale.mean() + 1e-10)
    iscale = 1.0 / np.clip(scale, 0.01, None)
    invl = (1.0 / (lengths + 1e-10)).astype(np.float32)

    # remap ids into padded vocab layout
    ids64 = ids.astype(np.int64)
    rid = (ids64 // VSH) * VSHP + (ids64 % VSH)  # [B, S] < 32768

    wdt = np.ascontiguousarray(wd.T).astype(ml_dtypes.bfloat16)  # [4096, 64]

    # W_re: result index = feat*3 + k  ->  W_k = W_re[:, k::3]  [64, 256]
    wret = np.zeros((2, 128, 3 * R), dtype=np.float32)
    for k in range(3):
        wkT = np.ascontiguousarray(wr[:, k::3].T)  # [256, 64]
        for kc in range(2):
            wret[kc, :, k * R : (k + 1) * R] = wkT[kc * 128 : (kc + 1) * 128, :]
    wret = wret.astype(ml_dtypes.bfloat16)
    biasr = np.tile(br[None, :], (128, 1)).astype(np.float32)
    identm = np.eye(128, dtype=np.float32)

    in_maps = []
    for c in range(NCORES):
        embt = np.zeros((HID, VSHP), dtype=ml_dtypes.bfloat16)
        embt[:, :VSH] = emb[c * VSH : (c + 1) * VSH, :].T.astype(ml_dtypes.bfloat16)
        # per-core padded rows
        rid_c = np.zeros((BPAD, S), dtype=np.int64)
        rid_c[:BSH] = rid[c * BSH : (c + 1) * BSH]
        # gather order: j = t*2048 + s*128 + r
        L = rid_c.reshape(NT, 128, S).transpose(0, 2, 1).reshape(-1)  # [BPAD*S]
        idx16 = L.reshape(-1, 16).T.astype(np.int16)  # [16, BPAD]
        idxsb = np.ascontiguousarray(np.tile(idx16, (8, 1)))  # [128, BPAD]

        auxc = np.zeros((128, 3 * NT), dtype=np.float32)
        for name_i, v in enumerate((invl, scale, iscale)):
            vc = np.ones(BPAD, dtype=np.float32)
            vc[:BSH] = v[c * BSH : (c + 1) * BSH]
            auxc[:, name_i * NT : (name_i + 1) * NT] = vc.reshape(NT, 128).T
        in_maps.append(
            dict(embt=embt, wdt=wdt, idx=idxsb, aux=auxc, wret=wret,
                 biasr=biasr, ident=identm)
        )
    return in_maps, lengths, scale, iscale, invl


def _patch_rows(result, text_embeddings, kgl2token, W_down, W_re, b_re,
                scale_all, iscale_all, invl_all):
    """Recompute rows containing any id==0 token exactly (host, numpy)."""
    ids = np.asarray(kgl2token)
    bad = np.nonzero((ids <= 0).any(axis=1))[0]
    if len(bad) == 0:
        return result
    emb = np.asarray(text_embeddings, dtype=np.float32)
    wd = np.asarray(W_down, dtype=np.float32)
    wr = np.asarray(W_re, dtype=np.float32)
    br = np.asarray(b_re, dtype=np.float32)
    for r in bad:
        tok_ids = ids[r].astype(np.int64)
        tok = emb[tok_ids] @ wd.T  # [S, R]
        mask = (tok_ids > 0).astype(np.float32)[:, None]
        length = mask.sum()
        masked = tok * mask
        mean = masked.sum(axis=0) / (length + 1e-10)
        sq_mean = (tok * tok * mask).sum(axis=0) / (length + 1e-10)
        mx = (masked + (1.0 - mask) * (-1e10)).max(axis=0)
        mn = (masked + (1.0 - mask) * (1e10)).min(axis=0)
        std = np.sqrt(np.clip(sq_mean - mean * mean, 1e-6, None))
        features = np.concatenate([mean, mx, mn, std])  # [256]
        scales = np.array([1.0, scale_all[r], iscale_all[r]], dtype=np.float32)
        flat = (features[:, None] * scales[None, :]).reshape(-1)  # [768]
        res = flat @ wr.T + br
        nrm = np.linalg.norm(res)
        result[r] = res / max(nrm, 1e-12)
    return result


def kernel(text_embeddings, kgl2token, W_down, W_re, b_re, _trace=False):
    nc = _get_nc()
    in_maps, lengths, scale, iscale, invl = _prepare(
        text_embeddings, kgl2token, W_down, W_re, b_re
    )
    r = run_bass_kernel_spmd(nc, in_maps, core_ids=list(range(NCORES)), trace=_trace)
    outs = [r.results[c]["out"][:BSH] for c in range(NCORES)]
    result = np.concatenate(outs, axis=0).astype(np.float32)
    result = _patch_rows(
        result, text_embeddings, kgl2token, W_down, W_re, b_re, scale, iscale, invl
    )
    if _trace:
        return result, r
    return result
